# revision 54
# baseline (speedup 1.0000x reference)
"""Trainium2 Bass kernel for nn_Attention_43181601194684.

Reference computation:
    h_last  = hidden[0, 1]                          # [B, H]
    proj    = einsum('blh,oh->blo', enc, W) + b     # [B, L, H]
    energies= einsum('bh,blh->bl', h_last, proj)    # [B, L]
    out     = softmax(energies, axis=1)[:, None, :] # [B, 1, L]

Algebraic simplification:
    energies[b, l] = (h_last[b] @ W) . enc[b, l] + (h_last[b] . bias)
The per-batch constant cancels inside the softmax, so the device kernel
computes   e[b, l] = v[b] . enc[b, l]   with v = h_last @ W, followed by a
numerically-stable softmax over l.  v = h_last @ W (the tiny [B,H]x[H,H]
GEMM) is done on the host.

Precision: enc and v are streamed in FP16 (host-converted); the products
accumulate into FP32 energies, exp runs in FP32 on the ACT engine, and the
exact cross-partition max correction + normalization happen on the host in
float64.  Measured output rel-err vs the fp32 reference is ~5e-3 (gate 2e-2).
FP16 halves the HBM traffic to 16.8 MiB/core.

Engine split (HW-measured pitches for one [128,512] unit): a fused DVE
scalar_tensor_tensor is 605 ns (1x); a plain DVE tensor_tensor multiply
hits the 2x mode at 335 ns; an ACT Copy-activation-with-accumulator
row-reduce is 628+186 ns.  Each chunk's rows are split ~7:9 between
(fused STT on DVE) and (DVE 2x multiply + ACT reduce) so both engines land
at ~58 us, just above the ~50 us DMA stream.  The Pool engine is left idle
on purpose: concurrent Pool work slows DVE ops ~2.5x.  Deep DMA prefetch
is also avoided (concurrent DMA SBUF writes slow every engine ~20%), so
in-flight chunks are capped by shallow tile pools on a single ring.

The wall time is  queue_start + total_bytes/DMA_BW + tail,  so the design
minimizes bytes, keeps every DMA a fully contiguous DRAM blob with fat
per-partition descriptor runs, limits in-flight chunks so the descriptor
round-robin doesn't starve the serial consumer, and puts SMALL chunks at
the two ends of the stream (early first STT, short post-last-byte tail).

Sharding: data-parallel over batch. 32 batches / 8 cores = 4 per core.
Each batch's first 3072 l-rows go through the DVE/ACT path in 1 MiB chunks
(l = off + p*j + k within a chunk of j rows at offset off):
batch 0 = [2,2,4,8,8], batches 1,2 = [8,8,8], batch 3 = [8,8,4,2,2].
The LAST 1024 l-rows of each batch are shipped host-transposed
([H, 1024] fp16, same bytes) and reduced on the otherwise-idle PE:
lhsT = v h-chunk [128,1], rhs = encT [128, 512 l] -> PSUM-accumulated
energies [2, 512] per batch at ~0.5 us of PE per 512 rows.
Output per batch: [128, 24] fp16 exp tile (DVE/ACT path) + [2, 512] fp32
raw energies (PE path); host applies exp for the PE rows, the row-max
correction, normalization, and un-permutation in float64.
"""

import numpy as np

B, L, H = 32, 4096, 512
N_CORES = 8
B_LOC = B // N_CORES  # 4
P = 128               # SBUF partitions
NCOL = L // P         # 32 energy columns per batch

NCOLR = 24            # l-rows per partition on the DVE/ACT path
LPE = 1024            # trailing l-rows per batch on the PE path
SCHEDS = {
    0: (2, 2, 4, 8, 8),
    1: (8, 8, 8),
    2: (8, 8, 8),
    3: (8, 8, 4, 2, 2),
}

_PROGRAM = None


def _build_program():
    """Build + compile the single-core Bass/Tile program (SPMD across 8 cores)."""
    from contextlib import ExitStack

    import concourse.bacc as bacc
    import concourse.mybir as mybir
    import concourse.tile as tile
    from concourse.masks import make_identity

    fp32 = mybir.dt.float32
    fp16 = mybir.dt.float16
    Alu = mybir.AluOpType
    Act = mybir.ActivationFunctionType

    nc = bacc.Bacc("TRN2", target_bir_lowering=False, debug=False,
                   num_devices=N_CORES)

    LREG = L - LPE
    enc = nc.dram_tensor("enc", [B_LOC, LREG, H], fp16, kind="ExternalInput")
    encT = nc.dram_tensor("encT", [B_LOC, P, H // P, LPE], fp16,
                          kind="ExternalInput")
    vr = nc.dram_tensor("vr", [B_LOC, P, H], fp16, kind="ExternalInput")
    vt = nc.dram_tensor("vt", [P, B_LOC, H // P], fp16,
                        kind="ExternalInput")
    probs = nc.dram_tensor("probs", [B_LOC, P, NCOLR], fp16,
                           kind="ExternalOutput")
    pprobs = nc.dram_tensor("pprobs", [B_LOC, 2, LPE // 2], fp32,
                            kind="ExternalOutput")
    mxs = nc.dram_tensor("mxs", [P, B_LOC], fp32, kind="ExternalOutput")

    # one rearranged view per chunk-row-count; chunk g of the k=j view
    # covers l in [g*128*j, (g+1)*128*j) with l = g*128*j + p*j + k
    enc_r = {
        j: enc.rearrange("b (g p k) h -> b g p k h", p=P, k=j)
        for j in (2, 4, 8)
    }

    with tile.TileContext(nc) as tc, ExitStack() as ctx:
        consts = ctx.enter_context(tc.tile_pool(name="consts", bufs=1))
        wpool = ctx.enter_context(tc.tile_pool(name="wpool", bufs=1))
        et8 = ctx.enter_context(tc.tile_pool(name="et8", bufs=4))
        et4 = ctx.enter_context(tc.tile_pool(name="et4", bufs=2))
        et2 = ctx.enter_context(tc.tile_pool(name="et2", bufs=4))
        scratch = ctx.enter_context(tc.tile_pool(name="scratch", bufs=3))
        dprod = ctx.enter_context(tc.tile_pool(name="dprod", bufs=8))
        pprod = ctx.enter_context(tc.tile_pool(name="pprod", bufs=6))
        aout = ctx.enter_context(tc.tile_pool(name="aout", bufs=3))
        epers = ctx.enter_context(tc.tile_pool(name="epers", bufs=1))
        small = ctx.enter_context(tc.tile_pool(name="small", bufs=2))
        pet = ctx.enter_context(tc.tile_pool(name="pet", bufs=1))
        pes = ctx.enter_context(tc.tile_pool(name="pes", bufs=1, space="PSUM"))
        etp = {2: et2, 4: et4, 8: et8}

        # split of each chunk's k-units across engines:
        # (fused STT on DVE) / (DVE 2x-TT + ACT copy-reduce) / (Pool TT +
        # ACT copy-reduce), balanced so DVE/ACT/Pool all land ~50us.
        NSPLIT = {8: (4, 4, 0), 4: (2, 2, 0), 2: (1, 1, 0)}

        # priority block: v (replicated fp16, 512 KiB) plus batch 0's head
        # chunks land first so the first STT fires as early as possible
        head = {}
        v_sb = {}
        with tc.high_priority():
            # the first STT's inputs go FIRST on the sync ring (it starts
            # draining ~2us before the scalar ring)
            head[0] = et2.tile([P, 2, H], fp16, tag="et2", name="hd0")
            nc.sync.dma_start(head[0][:], enc_r[2][0, 0])
            v_sb[0] = wpool.tile([P, H], fp16, tag="v0", name="v0")
            nc.sync.dma_start(v_sb[0][:], vr[0])
            head[1] = et2.tile([P, 2, H], fp16, tag="et2", name="hd1")
            nc.sync.dma_start(head[1][:], enc_r[2][0, 1])
            vt_sb = wpool.tile([P, B_LOC, H // P], fp16, tag="vt")
            nc.scalar.dma_start(vt_sb[:], vt[:])

        # ---- main stream: multiply+row-reduce split across DVE/ACT/Pool ----
        # All enc DMAs go on the sync ring IN ORDER so chunk completion
        # order matches consumption order (no descriptor round-robin skew).
        e_tiles = {}
        nbias_t = {}
        for bi in range(B_LOC):
            if bi not in v_sb:
                v_sb[bi] = wpool.tile([P, H], fp16, tag=f"v{bi}",
                                      name=f"v{bi}")
                nc.scalar.dma_start(v_sb[bi][:], vr[bi])
            sched = SCHEDS[bi]
            e_sb = epers.tile([P, NCOLR], fp32, tag=f"e{bi}",
                              name=f"e{bi}")
            e_tiles[bi] = e_sb
            m = 0
            off_rows = 0
            for cix, j in enumerate(sched):
                g = off_rows // j          # group index in the k=j view
                if bi == 0 and cix < 2:
                    et = head[cix]
                else:
                    et = etp[j].tile([P, j, H], fp16, tag=f"et{j}",
                                     name=f"et_{bi}_{cix}")
                    nc.sync.dma_start(et[:], enc_r[j][bi, g])
                if bi == 3 and cix >= 3:
                    n_stt, n_tt, n_pool = (j, 0, 0)   # STT-only tail
                else:
                    n_stt, n_tt, n_pool = NSPLIT[j]
                for k in range(j):
                    if k < n_stt:
                        # fused (enc * v) + row-sum on DVE (1x mode)
                        sc = scratch.tile([P, H], fp16, tag="ttr")
                        nc.vector.scalar_tensor_tensor(
                            out=sc[:], in0=et[:, k, :], scalar=1.0,
                            in1=v_sb[bi][:],
                            op0=Alu.mult, op1=Alu.mult,
                            accum_out=e_sb[:, m:m + 1],
                        )
                    else:
                        # 2x-mode multiply on DVE or Pool, row-reduce on ACT
                        # (Copy-activation with accumulator; the Copy table
                        # is loaded once and all Exps happen at the end)
                        if k < n_stt + n_tt:
                            prod = dprod.tile([P, H], fp16, tag="prod")
                            eng = nc.vector
                        else:
                            prod = pprod.tile([P, H], fp16, tag="pprod")
                            eng = nc.gpsimd
                        eng.tensor_tensor(out=prod[:], in0=et[:, k, :],
                                          in1=v_sb[bi][:], op=Alu.mult)
                        ao = aout.tile([P, H], fp16, tag="actout")
                        nc.scalar.activation(ao[:], prod[:], Act.Copy,
                                             bias=0.0, scale=1.0,
                                             accum_out=e_sb[:, m:m + 1])
                    m += 1
                off_rows += j

            # ---- PE path: energies for l in [3072, 4096) of this batch.
            # rhs tiles stream the host-transposed encT; lhsT is the v
            # h-chunk column; energies accumulate in PSUM over h-chunks.
            # (matmul out must sit at base partition 0 -> one tile/group;
            # the exp bias is a host-computed safe bound, no device max.)
            pe_g = [pes.tile([1, LPE // 2], fp32, tag=f"pe{bi}g{g}",
                             name=f"pe{bi}g{g}") for g in range(2)]
            # one 1 MiB DMA per batch with 8 KiB/partition runs (partition
            # p holds its 4 h-chunk rows contiguously); sync ring, in
            # program order AFTER this batch's regular chunks
            rt = pet.tile([P, H // P, LPE], fp16, tag="rt",
                          name=f"rt_{bi}")
            nc.sync.dma_start(rt[:], encT[bi])
            for hc in range(H // P):
                for g in range(2):
                    nc.tensor.matmul(
                        pe_g[g][:],
                        vt_sb[:, bi, hc:hc + 1],
                        rt[:, hc, g * (LPE // 2):(g + 1) * (LPE // 2)],
                        start=(hc == 0), stop=(hc == H // P - 1))
            nbias_t[("pe", bi)] = pe_g

            # negated per-partition row max: exp(e - rowmax_p) <= 1 is
            # overflow-safe; the host applies the cross-partition
            # correction exactly from the stored row maxes
            if bi == 0:
                mxall = small.tile([P, B_LOC], fp32, tag="mxall",
                                   name="mxall")
                nbias_t["all"] = mxall
            else:
                mxall = nbias_t["all"]
            nc.vector.tensor_reduce(mxall[:, bi:bi + 1], e_sb[:],
                                    axis=mybir.AxisListType.X,
                                    op=Alu.max, negate=True)
            nbias_t[bi] = mxall[:, bi:bi + 1]

        # ---- deferred exp tails, stage-major (ACT switches Copy->Exp once).
        # The normalizing divide happens on the host: p = exp(e - M) is
        # stored as-is and the host divides each batch by its sum.
        # PE-path ENERGIES bounce PSUM->SBUF on ACT (its Copy table is
        # already loaded; these run BEFORE every Exp so the activation
        # table switches exactly once), then to DRAM; the host applies
        # exp(e - M_b) for these rows in float64
        for bi in range(B_LOC):
            pe_g = nbias_t[("pe", bi)]
            for g in range(2):
                pp_t = epers.tile([1, LPE // 2], fp32, tag=f"pp{bi}g{g}",
                                  name=f"pp{bi}g{g}")
                nc.scalar.activation(pp_t[:], pe_g[g][:], Act.Copy,
                                     bias=0.0, scale=1.0)
                nc.gpsimd.dma_start(pprobs[bi, g], pp_t[:])
        for bi in range(B_LOC):
            p_t = epers.tile([P, NCOLR], fp16, tag=f"p{bi}",
                             name=f"p{bi}")
            if bi == 0:
                # row maxes are complete before the exps; store them first
                nc.sync.dma_start(mxs[:], nbias_t["all"][:])
            nc.scalar.activation(p_t[:], e_tiles[bi][:], Act.Exp,
                                 bias=nbias_t[bi], scale=1.0)
            # contiguous 8 KiB fp16 store (host normalizes in float64);
            # alternate between the idle SP and Pool DMA queues -- cross-
            # engine issue gets a real semaphore, unlike the ACT-ring variant
            r = nc.sync if bi % 2 == 0 else nc.gpsimd
            r.dma_start(probs[bi], p_t[:])

    nc.compile()
    return nc


def _get_program():
    global _PROGRAM
    if _PROGRAM is None:
        _PROGRAM = _build_program()
    return _PROGRAM


def _core_inputs(enc, v):
    """Per-core inputs: fp16 enc (first 3072 rows/batch), the last 1024
    rows host-transposed for the PE path, and v in both layouts."""
    enc16 = enc.astype(np.float16)
    v16 = v.astype(np.float16)
    in_maps = []
    for core in range(N_CORES):
        b0 = core * B_LOC
        vloc = v16[b0:b0 + B_LOC]
        v_rep = np.ascontiguousarray(
            np.broadcast_to(vloc[:, None, :], (B_LOC, P, H)))
        # vt[p, b, hc] = v[b, hc*128 + p]
        vt = np.ascontiguousarray(
            vloc.reshape(B_LOC, H // P, P).transpose(2, 0, 1))
        eloc = enc16[b0:b0 + B_LOC]
        in_maps.append({
            "enc": np.ascontiguousarray(eloc[:, :L - LPE]),
            "encT": np.ascontiguousarray(
                eloc[:, L - LPE:].transpose(0, 2, 1)
                .reshape(B_LOC, H // P, P, LPE).transpose(0, 2, 1, 3)),
            "vr": v_rep,
            "vt": vt,
        })
    return in_maps


def _assemble(res_list):
    """Per-core (probs, pprobs, mxs, pebias) -> full [B, 1, L].

    DVE/ACT path: column block [mc, mc+j) holds chunk (off, j) with
    l = off + p*j + k; tiles are exp(e - rowmax_p).  PE path: pprobs[g, c]
    is l = 3072 + g*512 + c as exp(e - C_b) with the safe bound C_b; its
    true max is C_b + ln(max value).  Host rescales everything to the
    global batch max in float64, then normalizes.
    """
    out = np.empty((B, L), dtype=np.float64)
    for core, (pr, ppr, mxs) in enumerate(res_list):
        rowmax = -np.asarray(mxs, dtype=np.float64)    # [P, B_LOC]
        for bl in range(B_LOC):
            b = core * B_LOC + bl
            pe = np.asarray(ppr[bl], dtype=np.float64)  # [2, 512] energies
            mb = max(rowmax[:, bl].max(), pe.max())
            w = np.exp(rowmax[:, bl] - mb)              # [P]
            scaled = np.asarray(pr[bl], dtype=np.float64) * w[:, None]
            mc = 0
            off = 0
            for j in SCHEDS[bl]:
                n = P * j
                out[b, off:off + n] = scaled[:, mc:mc + j].reshape(n)
                mc += j
                off += n
            out[b, L - LPE:] = np.exp(pe - mb).reshape(LPE)
            out[b] /= out[b].sum()
    return out[:, None, :].astype(np.float32)


def kernel(hidden, encoder_outputs, W, b):
    """Full-input entry point: shards across 8 NeuronCores, returns [B,1,L]."""
    from concourse.bass_utils import run_bass_kernel_spmd

    hidden = np.asarray(hidden, dtype=np.float32)
    enc = np.asarray(encoder_outputs, dtype=np.float32)
    W = np.asarray(W, dtype=np.float32)

    h_last = hidden[0, 1]          # == hidden[0].transpose(1,0,2)[:, -1, :]
    v = (h_last @ W).astype(np.float32)  # [B, H]; bias cancels in softmax

    nc = _get_program()
    in_maps = _core_inputs(enc, v)
    res = run_bass_kernel_spmd(nc, in_maps, list(range(N_CORES)))
    return _assemble([(res.results[i]["probs"], res.results[i]["pprobs"],
                       res.results[i]["mxs"])
                      for i in range(N_CORES)])


# revision 55
# speedup vs baseline: 1.1406x; 1.1406x over previous
"""Trainium2 Bass kernel for nn_Attention_43181601194684.

Reference computation:
    h_last  = hidden[0, 1]                          # [B, H]
    proj    = einsum('blh,oh->blo', enc, W) + b     # [B, L, H]
    energies= einsum('bh,blh->bl', h_last, proj)    # [B, L]
    out     = softmax(energies, axis=1)[:, None, :] # [B, 1, L]

Algebraic simplification:
    energies[b, l] = (h_last[b] @ W) . enc[b, l] + (h_last[b] . bias)
The per-batch constant cancels inside the softmax, so the device kernel
computes   e[b, l] = v[b] . enc[b, l]   with v = h_last @ W, followed by a
numerically-stable softmax over l.  v = h_last @ W (the tiny [B,H]x[H,H]
GEMM) is done on the host.

Precision: enc and v are streamed in FP16 (host-converted); the products
accumulate into FP32 energies, exp runs in FP32 on the ACT engine, and the
exact cross-partition max correction + normalization happen on the host in
float64.  Measured output rel-err vs the fp32 reference is ~5e-3 (gate 2e-2).
FP16 halves the HBM traffic to 16.8 MiB/core.

Engine split (HW-measured pitches for one [128,512] unit): a fused DVE
scalar_tensor_tensor is 605 ns (1x); a plain DVE tensor_tensor multiply
hits the 2x mode at 335 ns; an ACT Copy-activation-with-accumulator
row-reduce is 628+186 ns.  Each chunk's rows are split ~7:9 between
(fused STT on DVE) and (DVE 2x multiply + ACT reduce) so both engines land
at ~58 us, just above the ~50 us DMA stream.  The Pool engine is left idle
on purpose: concurrent Pool work slows DVE ops ~2.5x.  Deep DMA prefetch
is also avoided (concurrent DMA SBUF writes slow every engine ~20%), so
in-flight chunks are capped by shallow tile pools on a single ring.

The wall time is  queue_start + total_bytes/DMA_BW + tail,  so the design
minimizes bytes, keeps every DMA a fully contiguous DRAM blob with fat
per-partition descriptor runs, limits in-flight chunks so the descriptor
round-robin doesn't starve the serial consumer, and puts SMALL chunks at
the two ends of the stream (early first STT, short post-last-byte tail).

Sharding: data-parallel over batch. 32 batches / 8 cores = 4 per core.
Each batch's first 3072 l-rows go through the DVE/ACT path in 1 MiB chunks
(l = off + p*j + k within a chunk of j rows at offset off):
batch 0 = [2,2,4,8,8], batches 1,2 = [8,8,8], batch 3 = [8,8,4,2,2].
The LAST 1024 l-rows of each batch are shipped host-transposed
([H, 1024] fp16, same bytes) and reduced on the otherwise-idle PE:
lhsT = v h-chunk [128,1], rhs = encT [128, 512 l] -> PSUM-accumulated
energies [2, 512] per batch at ~0.5 us of PE per 512 rows.
Output per batch: [128, 24] fp16 exp tile (DVE/ACT path) + [2, 512] fp32
raw energies (PE path); host applies exp for the PE rows, the row-max
correction, normalization, and un-permutation in float64.
"""

import numpy as np

B, L, H = 32, 4096, 512
N_CORES = 8
B_LOC = B // N_CORES  # 4
P = 128               # SBUF partitions
NCOL = L // P         # 32 energy columns per batch

NCOLR = 24            # l-rows per partition on the DVE/ACT path
LPE = 1024            # trailing l-rows per batch on the PE path
SCHEDS = {
    0: (2, 2, 4, 8, 8),
    1: (8, 8, 8),
    2: (8, 8, 8),
    3: (8, 8, 4, 2, 2),
}

_PROGRAM = None


def _build_program():
    """Build + compile the single-core Bass/Tile program (SPMD across 8 cores)."""
    from contextlib import ExitStack

    import concourse.bacc as bacc
    import concourse.mybir as mybir
    import concourse.tile as tile
    from concourse.masks import make_identity

    fp32 = mybir.dt.float32
    fp16 = mybir.dt.float16
    Alu = mybir.AluOpType
    Act = mybir.ActivationFunctionType

    nc = bacc.Bacc("TRN2", target_bir_lowering=False, debug=False,
                   num_devices=N_CORES)

    LREG = L - LPE
    enc = nc.dram_tensor("enc", [B_LOC, LREG, H], fp16, kind="ExternalInput")
    encT = nc.dram_tensor("encT", [B_LOC, P, H // P, LPE], fp16,
                          kind="ExternalInput")
    vr = nc.dram_tensor("vr", [B_LOC, P, H], fp16, kind="ExternalInput")
    vt = nc.dram_tensor("vt", [P, B_LOC, H // P], fp16,
                        kind="ExternalInput")
    probs = nc.dram_tensor("probs", [B_LOC, P, NCOLR], fp16,
                           kind="ExternalOutput")
    pprobs = nc.dram_tensor("pprobs", [B_LOC, 2, LPE // 2], fp32,
                            kind="ExternalOutput")
    mxs = nc.dram_tensor("mxs", [P, B_LOC], fp32, kind="ExternalOutput")

    # one rearranged view per chunk-row-count; chunk g of the k=j view
    # covers l in [g*128*j, (g+1)*128*j) with l = g*128*j + p*j + k
    enc_r = {
        j: enc.rearrange("b (g p k) h -> b g p k h", p=P, k=j)
        for j in (2, 4, 8)
    }

    with tile.TileContext(nc) as tc, ExitStack() as ctx:
        consts = ctx.enter_context(tc.tile_pool(name="consts", bufs=1))
        wpool = ctx.enter_context(tc.tile_pool(name="wpool", bufs=1))
        et8 = ctx.enter_context(tc.tile_pool(name="et8", bufs=5))
        et4 = ctx.enter_context(tc.tile_pool(name="et4", bufs=2))
        et2 = ctx.enter_context(tc.tile_pool(name="et2", bufs=4))
        scratch = ctx.enter_context(tc.tile_pool(name="scratch", bufs=3))
        dprod = ctx.enter_context(tc.tile_pool(name="dprod", bufs=8))
        pprod = ctx.enter_context(tc.tile_pool(name="pprod", bufs=6))
        aout = ctx.enter_context(tc.tile_pool(name="aout", bufs=3))
        epers = ctx.enter_context(tc.tile_pool(name="epers", bufs=1))
        small = ctx.enter_context(tc.tile_pool(name="small", bufs=2))
        pet = ctx.enter_context(tc.tile_pool(name="pet", bufs=2))
        pes = ctx.enter_context(tc.tile_pool(name="pes", bufs=1, space="PSUM"))
        etp = {2: et2, 4: et4, 8: et8}

        # split of each chunk's k-units across engines:
        # (fused STT on DVE) / (DVE 2x-TT + ACT copy-reduce) / (Pool TT +
        # ACT copy-reduce), balanced so DVE/ACT/Pool all land ~50us.
        NSPLIT = {8: (4, 4, 0), 4: (2, 2, 0), 2: (1, 1, 0)}

        # priority block: v (replicated fp16, 512 KiB) plus batch 0's head
        # chunks land first so the first STT fires as early as possible
        head = {}
        v_sb = {}
        with tc.high_priority():
            # the first STT's inputs go FIRST on the sync ring (it starts
            # draining ~2us before the scalar ring)
            head[0] = et2.tile([P, 2, H], fp16, tag="et2", name="hd0")
            nc.sync.dma_start(head[0][:], enc_r[2][0, 0])
            v_sb[0] = wpool.tile([P, H], fp16, tag="v0", name="v0")
            nc.sync.dma_start(v_sb[0][:], vr[0])
            head[1] = et2.tile([P, 2, H], fp16, tag="et2", name="hd1")
            nc.sync.dma_start(head[1][:], enc_r[2][0, 1])
            vt_sb = wpool.tile([P, B_LOC, H // P], fp16, tag="vt")
            nc.scalar.dma_start(vt_sb[:], vt[:])

        # ---- main stream: multiply+row-reduce split across DVE/ACT/Pool ----
        # All enc DMAs go on the sync ring IN ORDER so chunk completion
        # order matches consumption order (no descriptor round-robin skew).
        e_tiles = {}
        nbias_t = {}
        for bi in range(B_LOC):
            if bi not in v_sb:
                v_sb[bi] = wpool.tile([P, H], fp16, tag=f"v{bi}",
                                      name=f"v{bi}")
                nc.scalar.dma_start(v_sb[bi][:], vr[bi])
            sched = SCHEDS[bi]
            e_sb = epers.tile([P, NCOLR], fp32, tag=f"e{bi}",
                              name=f"e{bi}")
            e_tiles[bi] = e_sb
            m = 0
            off_rows = 0
            for cix, j in enumerate(sched):
                g = off_rows // j          # group index in the k=j view
                if bi == 0 and cix < 2:
                    et = head[cix]
                else:
                    et = etp[j].tile([P, j, H], fp16, tag=f"et{j}",
                                     name=f"et_{bi}_{cix}")
                    nc.sync.dma_start(et[:], enc_r[j][bi, g])
                if bi == 3 and cix >= 3:
                    n_stt, n_tt, n_pool = (j, 0, 0)   # STT-only tail
                else:
                    n_stt, n_tt, n_pool = NSPLIT[j]
                for k in range(j):
                    if k < n_stt:
                        # fused (enc * v) + row-sum on DVE (1x mode)
                        sc = scratch.tile([P, H], fp16, tag="ttr")
                        nc.vector.scalar_tensor_tensor(
                            out=sc[:], in0=et[:, k, :], scalar=1.0,
                            in1=v_sb[bi][:],
                            op0=Alu.mult, op1=Alu.mult,
                            accum_out=e_sb[:, m:m + 1],
                        )
                    else:
                        # 2x-mode multiply on DVE or Pool, row-reduce on ACT
                        # (Copy-activation with accumulator; the Copy table
                        # is loaded once and all Exps happen at the end)
                        if k < n_stt + n_tt:
                            prod = dprod.tile([P, H], fp16, tag="prod")
                            eng = nc.vector
                        else:
                            prod = pprod.tile([P, H], fp16, tag="pprod")
                            eng = nc.gpsimd
                        eng.tensor_tensor(out=prod[:], in0=et[:, k, :],
                                          in1=v_sb[bi][:], op=Alu.mult)
                        ao = aout.tile([P, H], fp16, tag="actout")
                        nc.scalar.activation(ao[:], prod[:], Act.Copy,
                                             bias=0.0, scale=1.0,
                                             accum_out=e_sb[:, m:m + 1])
                    m += 1
                off_rows += j

            # ---- PE path: energies for l in [3072, 4096) of this batch.
            # rhs tiles stream the host-transposed encT; lhsT is the v
            # h-chunk column; energies accumulate in PSUM over h-chunks.
            # (matmul out must sit at base partition 0 -> one tile/group;
            # the exp bias is a host-computed safe bound, no device max.)
            pe_g = [pes.tile([1, LPE // 2], fp32, tag=f"pe{bi}g{g}",
                             name=f"pe{bi}g{g}") for g in range(2)]
            # one 1 MiB DMA per batch with 8 KiB/partition runs (partition
            # p holds its 4 h-chunk rows contiguously); sync ring, in
            # program order AFTER this batch's regular chunks
            rt = pet.tile([P, H // P, LPE], fp16, tag="rt",
                          name=f"rt_{bi}")
            nc.sync.dma_start(rt[:], encT[bi])
            for hc in range(H // P):
                for g in range(2):
                    nc.tensor.matmul(
                        pe_g[g][:],
                        vt_sb[:, bi, hc:hc + 1],
                        rt[:, hc, g * (LPE // 2):(g + 1) * (LPE // 2)],
                        start=(hc == 0), stop=(hc == H // P - 1))
            nbias_t[("pe", bi)] = pe_g

            # negated per-partition row max: exp(e - rowmax_p) <= 1 is
            # overflow-safe; the host applies the cross-partition
            # correction exactly from the stored row maxes
            if bi == 0:
                mxall = small.tile([P, B_LOC], fp32, tag="mxall",
                                   name="mxall")
                nbias_t["all"] = mxall
            else:
                mxall = nbias_t["all"]
            nc.vector.tensor_reduce(mxall[:, bi:bi + 1], e_sb[:],
                                    axis=mybir.AxisListType.X,
                                    op=Alu.max, negate=True)
            nbias_t[bi] = mxall[:, bi:bi + 1]

        # ---- deferred exp tails, stage-major (ACT switches Copy->Exp once).
        # The normalizing divide happens on the host: p = exp(e - M) is
        # stored as-is and the host divides each batch by its sum.
        # PE-path ENERGIES bounce PSUM->SBUF on ACT (its Copy table is
        # already loaded; these run BEFORE every Exp so the activation
        # table switches exactly once), then to DRAM; the host applies
        # exp(e - M_b) for these rows in float64
        for bi in range(B_LOC):
            pe_g = nbias_t[("pe", bi)]
            for g in range(2):
                pp_t = epers.tile([1, LPE // 2], fp32, tag=f"pp{bi}g{g}",
                                  name=f"pp{bi}g{g}")
                nc.scalar.activation(pp_t[:], pe_g[g][:], Act.Copy,
                                     bias=0.0, scale=1.0)
                nc.gpsimd.dma_start(pprobs[bi, g], pp_t[:])
        for bi in range(B_LOC):
            p_t = epers.tile([P, NCOLR], fp16, tag=f"p{bi}",
                             name=f"p{bi}")
            if bi == 0:
                # row maxes are complete before the exps; store them first
                nc.sync.dma_start(mxs[:], nbias_t["all"][:])
            nc.scalar.activation(p_t[:], e_tiles[bi][:], Act.Exp,
                                 bias=nbias_t[bi], scale=1.0)
            # contiguous 8 KiB fp16 store (host normalizes in float64);
            # alternate between the idle SP and Pool DMA queues -- cross-
            # engine issue gets a real semaphore, unlike the ACT-ring variant
            r = nc.sync if bi % 2 == 0 else nc.gpsimd
            r.dma_start(probs[bi], p_t[:])

    nc.compile()
    return nc


def _get_program():
    global _PROGRAM
    if _PROGRAM is None:
        _PROGRAM = _build_program()
    return _PROGRAM


def _core_inputs(enc, v):
    """Per-core inputs: fp16 enc (first 3072 rows/batch), the last 1024
    rows host-transposed for the PE path, and v in both layouts."""
    enc16 = enc.astype(np.float16)
    v16 = v.astype(np.float16)
    in_maps = []
    for core in range(N_CORES):
        b0 = core * B_LOC
        vloc = v16[b0:b0 + B_LOC]
        v_rep = np.ascontiguousarray(
            np.broadcast_to(vloc[:, None, :], (B_LOC, P, H)))
        # vt[p, b, hc] = v[b, hc*128 + p]
        vt = np.ascontiguousarray(
            vloc.reshape(B_LOC, H // P, P).transpose(2, 0, 1))
        eloc = enc16[b0:b0 + B_LOC]
        in_maps.append({
            "enc": np.ascontiguousarray(eloc[:, :L - LPE]),
            "encT": np.ascontiguousarray(
                eloc[:, L - LPE:].transpose(0, 2, 1)
                .reshape(B_LOC, H // P, P, LPE).transpose(0, 2, 1, 3)),
            "vr": v_rep,
            "vt": vt,
        })
    return in_maps


def _assemble(res_list):
    """Per-core (probs, pprobs, mxs, pebias) -> full [B, 1, L].

    DVE/ACT path: column block [mc, mc+j) holds chunk (off, j) with
    l = off + p*j + k; tiles are exp(e - rowmax_p).  PE path: pprobs[g, c]
    is l = 3072 + g*512 + c as exp(e - C_b) with the safe bound C_b; its
    true max is C_b + ln(max value).  Host rescales everything to the
    global batch max in float64, then normalizes.
    """
    out = np.empty((B, L), dtype=np.float64)
    for core, (pr, ppr, mxs) in enumerate(res_list):
        rowmax = -np.asarray(mxs, dtype=np.float64)    # [P, B_LOC]
        for bl in range(B_LOC):
            b = core * B_LOC + bl
            pe = np.asarray(ppr[bl], dtype=np.float64)  # [2, 512] energies
            mb = max(rowmax[:, bl].max(), pe.max())
            w = np.exp(rowmax[:, bl] - mb)              # [P]
            scaled = np.asarray(pr[bl], dtype=np.float64) * w[:, None]
            mc = 0
            off = 0
            for j in SCHEDS[bl]:
                n = P * j
                out[b, off:off + n] = scaled[:, mc:mc + j].reshape(n)
                mc += j
                off += n
            out[b, L - LPE:] = np.exp(pe - mb).reshape(LPE)
            out[b] /= out[b].sum()
    return out[:, None, :].astype(np.float32)


def kernel(hidden, encoder_outputs, W, b):
    """Full-input entry point: shards across 8 NeuronCores, returns [B,1,L]."""
    from concourse.bass_utils import run_bass_kernel_spmd

    hidden = np.asarray(hidden, dtype=np.float32)
    enc = np.asarray(encoder_outputs, dtype=np.float32)
    W = np.asarray(W, dtype=np.float32)

    h_last = hidden[0, 1]          # == hidden[0].transpose(1,0,2)[:, -1, :]
    v = (h_last @ W).astype(np.float32)  # [B, H]; bias cancels in softmax

    nc = _get_program()
    in_maps = _core_inputs(enc, v)
    res = run_bass_kernel_spmd(nc, in_maps, list(range(N_CORES)))
    return _assemble([(res.results[i]["probs"], res.results[i]["pprobs"],
                       res.results[i]["mxs"])
                      for i in range(N_CORES)])


# revision 56
# speedup vs baseline: 1.1547x; 1.0123x over previous
"""Trainium2 Bass kernel for nn_Attention_43181601194684.

Reference computation:
    h_last  = hidden[0, 1]                          # [B, H]
    proj    = einsum('blh,oh->blo', enc, W) + b     # [B, L, H]
    energies= einsum('bh,blh->bl', h_last, proj)    # [B, L]
    out     = softmax(energies, axis=1)[:, None, :] # [B, 1, L]

Algebraic simplification:
    energies[b, l] = (h_last[b] @ W) . enc[b, l] + (h_last[b] . bias)
The per-batch constant cancels inside the softmax, so the device kernel
computes   e[b, l] = v[b] . enc[b, l]   with v = h_last @ W, followed by a
numerically-stable softmax over l.  v = h_last @ W (the tiny [B,H]x[H,H]
GEMM) is done on the host.

Precision: enc and v are streamed in FP16 (host-converted); the products
accumulate into FP32 energies, exp runs in FP32 on the ACT engine, and the
exact cross-partition max correction + normalization happen on the host in
float64.  Measured output rel-err vs the fp32 reference is ~5e-3 (gate 2e-2).
FP16 halves the HBM traffic to 16.8 MiB/core.

Engine split (HW-measured pitches for one [128,512] unit): a fused DVE
scalar_tensor_tensor is 605 ns (1x); a plain DVE tensor_tensor multiply
hits the 2x mode at 335 ns; an ACT Copy-activation-with-accumulator
row-reduce is 628+186 ns.  Each chunk's rows are split 4:4 between
(fused STT on DVE) and (DVE 2x multiply + ACT reduce); with the PE path
taking a quarter of all rows, DVE lands ~46 us and ACT ~45 us, under the
~51 us DMA stream which paces the kernel.  The Pool engine is left idle
on purpose: concurrent Pool work slows DVE ops ~2.5x.  Deep DMA prefetch
is also avoided (concurrent DMA SBUF writes slow every engine ~20%), so
in-flight chunks are capped by shallow tile pools on a single ring.

The wall time is  queue_start + total_bytes/DMA_BW + tail,  so the design
minimizes bytes, keeps every DMA a fully contiguous DRAM blob with fat
per-partition descriptor runs, limits in-flight chunks so the descriptor
round-robin doesn't starve the serial consumer, and puts SMALL chunks at
the two ends of the stream (early first STT, short post-last-byte tail).

Sharding: data-parallel over batch. 32 batches / 8 cores = 4 per core.
Each batch's first 3072 l-rows go through the DVE/ACT path in 1 MiB chunks
(l = off + p*j + k within a chunk of j rows at offset off):
batch 0 = [2,2,4,8,8], batches 1,2 = [8,8,8], batch 3 = [8,8,4,2,2].
The LAST 1024 l-rows of each batch are shipped host-transposed
([H, 1024] fp16, same bytes) and reduced on the otherwise-idle PE:
lhsT = v h-chunk [128,1], rhs = encT [128, 512 l] -> PSUM-accumulated
energies [2, 512] per batch at ~0.5 us of PE per 512 rows.
Output per batch: [128, 24] fp16 exp tile (DVE/ACT path) + [2, 512] fp32
raw energies (PE path); host applies exp for the PE rows, the row-max
correction, normalization, and un-permutation in float64.
"""

import numpy as np

B, L, H = 32, 4096, 512
N_CORES = 8
B_LOC = B // N_CORES  # 4
P = 128               # SBUF partitions
NCOL = L // P         # 32 energy columns per batch

NCOLR = 24            # l-rows per partition on the DVE/ACT path
LPE = 1024            # trailing l-rows per batch on the PE path
SCHEDS = {
    0: (2, 2, 4, 8, 8),
    1: (8, 8, 8),
    2: (8, 8, 8),
    3: (8, 8, 4, 2, 2),
}

_PROGRAM = None


def _build_program():
    """Build + compile the single-core Bass/Tile program (SPMD across 8 cores)."""
    from contextlib import ExitStack

    import concourse.bacc as bacc
    import concourse.mybir as mybir
    import concourse.tile as tile
    from concourse.masks import make_identity

    fp32 = mybir.dt.float32
    fp16 = mybir.dt.float16
    Alu = mybir.AluOpType
    Act = mybir.ActivationFunctionType

    nc = bacc.Bacc("TRN2", target_bir_lowering=False, debug=False,
                   num_devices=N_CORES)

    LREG = L - LPE
    enc = nc.dram_tensor("enc", [B_LOC, LREG, H], fp16, kind="ExternalInput")
    encT = nc.dram_tensor("encT", [B_LOC, P, H // P, LPE], fp16,
                          kind="ExternalInput")
    vr = nc.dram_tensor("vr", [B_LOC, P, H], fp16, kind="ExternalInput")
    vt = nc.dram_tensor("vt", [P, B_LOC, H // P], fp16,
                        kind="ExternalInput")
    probs = nc.dram_tensor("probs", [B_LOC, P, NCOLR], fp16,
                           kind="ExternalOutput")
    pprobs = nc.dram_tensor("pprobs", [B_LOC, 2, LPE // 2], fp32,
                            kind="ExternalOutput")
    mxs = nc.dram_tensor("mxs", [P, B_LOC], fp32, kind="ExternalOutput")

    # one rearranged view per chunk-row-count; chunk g of the k=j view
    # covers l in [g*128*j, (g+1)*128*j) with l = g*128*j + p*j + k
    enc_r = {
        j: enc.rearrange("b (g p k) h -> b g p k h", p=P, k=j)
        for j in (2, 4, 8)
    }

    with tile.TileContext(nc) as tc, ExitStack() as ctx:
        consts = ctx.enter_context(tc.tile_pool(name="consts", bufs=1))
        wpool = ctx.enter_context(tc.tile_pool(name="wpool", bufs=1))
        et8 = ctx.enter_context(tc.tile_pool(name="et8", bufs=5))
        et4 = ctx.enter_context(tc.tile_pool(name="et4", bufs=2))
        et2 = ctx.enter_context(tc.tile_pool(name="et2", bufs=4))
        scratch = ctx.enter_context(tc.tile_pool(name="scratch", bufs=3))
        dprod = ctx.enter_context(tc.tile_pool(name="dprod", bufs=8))
        pprod = ctx.enter_context(tc.tile_pool(name="pprod", bufs=6))
        aout = ctx.enter_context(tc.tile_pool(name="aout", bufs=3))
        epers = ctx.enter_context(tc.tile_pool(name="epers", bufs=1))
        small = ctx.enter_context(tc.tile_pool(name="small", bufs=2))
        pet = ctx.enter_context(tc.tile_pool(name="pet", bufs=2))
        pes = ctx.enter_context(tc.tile_pool(name="pes", bufs=1, space="PSUM"))
        etp = {2: et2, 4: et4, 8: et8}

        # split of each chunk's k-units across engines:
        # (fused STT on DVE) / (DVE 2x-TT + ACT copy-reduce) / (Pool TT +
        # ACT copy-reduce), balanced so DVE/ACT/Pool all land ~50us.
        NSPLIT = {8: (4, 4, 0), 4: (2, 2, 0), 2: (1, 1, 0)}

        # priority block: v (replicated fp16, 512 KiB) plus batch 0's head
        # chunks land first so the first STT fires as early as possible
        head = {}
        v_sb = {}
        with tc.high_priority():
            # the first STT's inputs go FIRST on the sync ring (it starts
            # draining ~2us before the scalar ring)
            head[0] = et2.tile([P, 2, H], fp16, tag="et2", name="hd0")
            nc.sync.dma_start(head[0][:], enc_r[2][0, 0])
            v_sb[0] = wpool.tile([P, H], fp16, tag="v0", name="v0")
            nc.sync.dma_start(v_sb[0][:], vr[0])
            head[1] = et2.tile([P, 2, H], fp16, tag="et2", name="hd1")
            nc.sync.dma_start(head[1][:], enc_r[2][0, 1])
            vt_sb = wpool.tile([P, B_LOC, H // P], fp16, tag="vt")
            nc.scalar.dma_start(vt_sb[:], vt[:])

        # ---- main stream: multiply+row-reduce split across DVE/ACT/Pool ----
        # All enc DMAs go on the sync ring IN ORDER so chunk completion
        # order matches consumption order (no descriptor round-robin skew).
        e_tiles = {}
        nbias_t = {}
        for bi in range(B_LOC):
            if bi not in v_sb:
                v_sb[bi] = wpool.tile([P, H], fp16, tag=f"v{bi}",
                                      name=f"v{bi}")
                nc.scalar.dma_start(v_sb[bi][:], vr[bi])
            sched = SCHEDS[bi]
            e_sb = epers.tile([P, NCOLR], fp32, tag=f"e{bi}",
                              name=f"e{bi}")
            e_tiles[bi] = e_sb
            m = 0
            off_rows = 0
            for cix, j in enumerate(sched):
                g = off_rows // j          # group index in the k=j view
                if bi == 0 and cix < 2:
                    et = head[cix]
                else:
                    et = etp[j].tile([P, j, H], fp16, tag=f"et{j}",
                                     name=f"et_{bi}_{cix}")
                    nc.sync.dma_start(et[:], enc_r[j][bi, g])
                if bi == 3 and cix >= 3:
                    n_stt, n_tt, n_pool = (j, 0, 0)   # STT-only tail
                else:
                    n_stt, n_tt, n_pool = NSPLIT[j]
                for k in range(j):
                    if k < n_stt:
                        # fused (enc * v) + row-sum on DVE (1x mode)
                        sc = scratch.tile([P, H], fp16, tag="ttr")
                        nc.vector.scalar_tensor_tensor(
                            out=sc[:], in0=et[:, k, :], scalar=1.0,
                            in1=v_sb[bi][:],
                            op0=Alu.mult, op1=Alu.mult,
                            accum_out=e_sb[:, m:m + 1],
                        )
                    else:
                        # 2x-mode multiply on DVE or Pool, row-reduce on ACT
                        # (Copy-activation with accumulator; the Copy table
                        # is loaded once and all Exps happen at the end)
                        if k < n_stt + n_tt:
                            prod = dprod.tile([P, H], fp16, tag="prod")
                            eng = nc.vector
                        else:
                            prod = pprod.tile([P, H], fp16, tag="pprod")
                            eng = nc.gpsimd
                        eng.tensor_tensor(out=prod[:], in0=et[:, k, :],
                                          in1=v_sb[bi][:], op=Alu.mult)
                        ao = aout.tile([P, H], fp16, tag="actout")
                        nc.scalar.activation(ao[:], prod[:], Act.Copy,
                                             bias=0.0, scale=1.0,
                                             accum_out=e_sb[:, m:m + 1])
                    m += 1
                off_rows += j

            # ---- PE path: energies for l in [3072, 4096) of this batch.
            # rhs tiles stream the host-transposed encT; lhsT is the v
            # h-chunk column; energies accumulate in PSUM over h-chunks.
            # (matmul out must sit at base partition 0 -> one tile/group;
            # the exp bias is a host-computed safe bound, no device max.)
            pe_g = [pes.tile([1, LPE // 2], fp32, tag=f"pe{bi}g{g}",
                             name=f"pe{bi}g{g}") for g in range(2)]
            # one 1 MiB DMA per batch with 8 KiB/partition runs (partition
            # p holds its 4 h-chunk rows contiguously); sync ring, in
            # program order AFTER this batch's regular chunks
            rt = pet.tile([P, H // P, LPE], fp16, tag="rt",
                          name=f"rt_{bi}")
            nc.sync.dma_start(rt[:], encT[bi])
            for hc in range(H // P):
                for g in range(2):
                    nc.tensor.matmul(
                        pe_g[g][:],
                        vt_sb[:, bi, hc:hc + 1],
                        rt[:, hc, g * (LPE // 2):(g + 1) * (LPE // 2)],
                        start=(hc == 0), stop=(hc == H // P - 1))
            nbias_t[("pe", bi)] = pe_g

            # negated per-partition row max: exp(e - rowmax_p) <= 1 is
            # overflow-safe; the host applies the cross-partition
            # correction exactly from the stored row maxes
            if bi == 0:
                mxall = small.tile([P, B_LOC], fp32, tag="mxall",
                                   name="mxall")
                nbias_t["all"] = mxall
            else:
                mxall = nbias_t["all"]
            nc.vector.tensor_reduce(mxall[:, bi:bi + 1], e_sb[:],
                                    axis=mybir.AxisListType.X,
                                    op=Alu.max, negate=True)
            nbias_t[bi] = mxall[:, bi:bi + 1]

        # ---- deferred exp tails, stage-major (ACT switches Copy->Exp once).
        # The normalizing divide happens on the host: p = exp(e - M) is
        # stored as-is and the host divides each batch by its sum.
        # PE-path ENERGIES bounce PSUM->SBUF on ACT (its Copy table is
        # already loaded; these run BEFORE every Exp so the activation
        # table switches exactly once), then to DRAM; the host applies
        # exp(e - M_b) for these rows in float64
        for bi in range(B_LOC):
            pe_g = nbias_t[("pe", bi)]
            for g in range(2):
                pp_t = epers.tile([1, LPE // 2], fp32, tag=f"pp{bi}g{g}",
                                  name=f"pp{bi}g{g}")
                nc.scalar.activation(pp_t[:], pe_g[g][:], Act.Copy,
                                     bias=0.0, scale=1.0)
                nc.gpsimd.dma_start(pprobs[bi, g], pp_t[:])
        for bi in range(B_LOC):
            p_t = epers.tile([P, NCOLR], fp16, tag=f"p{bi}",
                             name=f"p{bi}")
            if bi == 0:
                # row maxes are complete before the exps; store them first
                nc.sync.dma_start(mxs[:], nbias_t["all"][:])
            nc.scalar.activation(p_t[:], e_tiles[bi][:], Act.Exp,
                                 bias=nbias_t[bi], scale=1.0)
            # contiguous 8 KiB fp16 store (host normalizes in float64);
            # alternate between the idle SP and Pool DMA queues -- cross-
            # engine issue gets a real semaphore, unlike the ACT-ring variant
            r = nc.sync if bi % 2 == 0 else nc.gpsimd
            r.dma_start(probs[bi], p_t[:])

    nc.compile()
    return nc


def _get_program():
    global _PROGRAM
    if _PROGRAM is None:
        _PROGRAM = _build_program()
    return _PROGRAM


def _core_inputs(enc, v):
    """Per-core inputs: fp16 enc (first 3072 rows/batch), the last 1024
    rows host-transposed for the PE path, and v in both layouts."""
    enc16 = enc.astype(np.float16)
    v16 = v.astype(np.float16)
    in_maps = []
    for core in range(N_CORES):
        b0 = core * B_LOC
        vloc = v16[b0:b0 + B_LOC]
        v_rep = np.ascontiguousarray(
            np.broadcast_to(vloc[:, None, :], (B_LOC, P, H)))
        # vt[p, b, hc] = v[b, hc*128 + p]
        vt = np.ascontiguousarray(
            vloc.reshape(B_LOC, H // P, P).transpose(2, 0, 1))
        eloc = enc16[b0:b0 + B_LOC]
        in_maps.append({
            "enc": np.ascontiguousarray(eloc[:, :L - LPE]),
            "encT": np.ascontiguousarray(
                eloc[:, L - LPE:].transpose(0, 2, 1)
                .reshape(B_LOC, H // P, P, LPE).transpose(0, 2, 1, 3)),
            "vr": v_rep,
            "vt": vt,
        })
    return in_maps


def _assemble(res_list):
    """Per-core (probs, pprobs, mxs, pebias) -> full [B, 1, L].

    DVE/ACT path: column block [mc, mc+j) holds chunk (off, j) with
    l = off + p*j + k; tiles are exp(e - rowmax_p).  PE path: pprobs[g, c]
    is l = 3072 + g*512 + c as exp(e - C_b) with the safe bound C_b; its
    true max is C_b + ln(max value).  Host rescales everything to the
    global batch max in float64, then normalizes.
    """
    out = np.empty((B, L), dtype=np.float64)
    for core, (pr, ppr, mxs) in enumerate(res_list):
        rowmax = -np.asarray(mxs, dtype=np.float64)    # [P, B_LOC]
        for bl in range(B_LOC):
            b = core * B_LOC + bl
            pe = np.asarray(ppr[bl], dtype=np.float64)  # [2, 512] energies
            mb = max(rowmax[:, bl].max(), pe.max())
            w = np.exp(rowmax[:, bl] - mb)              # [P]
            scaled = np.asarray(pr[bl], dtype=np.float64) * w[:, None]
            mc = 0
            off = 0
            for j in SCHEDS[bl]:
                n = P * j
                out[b, off:off + n] = scaled[:, mc:mc + j].reshape(n)
                mc += j
                off += n
            out[b, L - LPE:] = np.exp(pe - mb).reshape(LPE)
            out[b] /= out[b].sum()
    return out[:, None, :].astype(np.float32)


def kernel(hidden, encoder_outputs, W, b):
    """Full-input entry point: shards across 8 NeuronCores, returns [B,1,L]."""
    from concourse.bass_utils import run_bass_kernel_spmd

    hidden = np.asarray(hidden, dtype=np.float32)
    enc = np.asarray(encoder_outputs, dtype=np.float32)
    W = np.asarray(W, dtype=np.float32)

    h_last = hidden[0, 1]          # == hidden[0].transpose(1,0,2)[:, -1, :]
    v = (h_last @ W).astype(np.float32)  # [B, H]; bias cancels in softmax

    nc = _get_program()
    in_maps = _core_inputs(enc, v)
    res = run_bass_kernel_spmd(nc, in_maps, list(range(N_CORES)))
    return _assemble([(res.results[i]["probs"], res.results[i]["pprobs"],
                       res.results[i]["mxs"])
                      for i in range(N_CORES)])
